# revision 51
# baseline (speedup 1.0000x reference)
"""CDMF segment-reduce kernel for 8 Trainium2 NeuronCores.

Strategy
--------
Host (cheap, index-only + layout + one big gather):
  * stable-sort rows by user id; cut the 100k rows into 8 shards at user
    boundaries ("expert-style sharding of user segments") so each core owns a
    disjoint user range -> no cross-core reduction needed at all.
  * pad every shard to NT*128 rows (mask=0 rows contribute exactly 0).
  * pre-gather q = item_emb[items] per shard (bf16).
  * store R transposed per tile as bf16 with the feature (d) axis on SBUF
    partitions, two seq-steps stacked per 128-partition group, so the PE can
    contract against w directly (Z = R.w as 25 tiny matmuls per tile).
  * build per-tile one-hot matrices (fp8, exact 0/1) mapping the 128 rows of
    a tile to the user-slots of a PSUM "bank" (bank b = users first seen in
    tile b), packed [seg_prev|seg_own|gat_prev|gat_own] in one 512B row.

Device (one SPMD program on 8 cores, DMA-bound ~2.5us/tile):
  * stream R2 tiles [128, 25, 128] bf16; 25 PE matmuls against w2 [128, 2]
    produce Z [128 rows, 50] in PSUM (f32 accum)
  * one DVE scalar_tensor_tensor: wt = sum_s max(Z, tau) * (mask*cnt)
  * ACT builds X = [wt*q | wt] (bf16)
  * PE one-hot matmuls accumulate per-user [sum wt*q | sum wt] into PSUM
    banks; ACT flushes each bank to SBUF (bf16)
  * interleaved gather: transposed one-hot matmuls read num/den back per row,
    DVE multiplies by q and row-reduces; single reciprocal at the end.
"""

import numpy as np
import ml_dtypes

import concourse.bass as bass
import concourse.tile as tile
from concourse import bacc, mybir
from concourse.bass_utils import run_bass_kernel_spmd

N_CORES = 8
TAU = 0.01
S = 50          # seq_len
D = 64          # n_features
E = 128         # emb_dim
SP = S // 2     # s-pairs per tile
F32 = mybir.dt.float32
BF16 = mybir.dt.bfloat16
FP8 = mybir.dt.float8e4

NP_BF16 = ml_dtypes.bfloat16
NP_FP8 = ml_dtypes.float8_e4m3fn


# ----------------------------------------------------------------------------
# host-side preprocessing
# ----------------------------------------------------------------------------

def _preprocess(users, items, R_ui, mask, w, item_emb):
    n = users.shape[0]
    perm = np.argsort(users, kind="stable")
    users_s = users[perm]

    # shard cuts at user boundaries
    cuts = [0]
    for c in range(1, N_CORES):
        t = round(c * n / N_CORES)
        while 0 < t < n and users_s[t] == users_s[t - 1]:
            t += 1
        cuts.append(min(t, n))
    cuts.append(n)
    sizes = [cuts[c + 1] - cuts[c] for c in range(N_CORES)]
    NT = max(2, int(np.ceil(max(sizes) / 128)))
    NPAD = NT * 128

    q_full = item_emb[items]  # [n, E]

    # w2 [128, 2]: column 0 contracts even s (partitions 0:64), column 1 odd s
    w2 = np.zeros((128, 2), NP_BF16)
    w2[0:D, 0] = w
    w2[D:128, 1] = w

    in_maps = []
    metas = []
    for c in range(N_CORES):
        lo, hi = cuts[c], cuts[c + 1]
        nc_rows = hi - lo
        p = perm[lo:hi]

        Rp = np.zeros((NPAD, S, D), np.float32)
        Rp[:nc_rows] = R_ui[p]
        # [NT,128,SP,2,D] -> [(2,D)=128, NT, SP, 128row]
        R2 = np.ascontiguousarray(
            Rp.reshape(NT, 128, SP, 2, D).transpose(3, 4, 0, 2, 1)
            .reshape(128, NT, SP, 128).astype(NP_FP8)
        )

        mk = np.zeros((NPAD, S), np.float32)
        mk[:nc_rows] = mask[p]
        maskw = mk.reshape(NT, 128, S).transpose(1, 0, 2)
        cntw = maskw.sum(-1)  # [128, NT]
        # fast path (alpha=beta=gamma=1): wt = (sum_s mask*Wv) * cnt, so
        # pre-scaling the mask by cnt lets one fused op produce wt directly
        maskc = np.ascontiguousarray(
            (maskw * cntw[:, :, None]).astype(NP_BF16))
        maskw = np.ascontiguousarray(maskw.astype(NP_BF16))
        cntw = np.ascontiguousarray(cntw.astype(np.float32))

        # q with a ones column appended so ONE ACT op builds X = [wt*q | wt]
        qp = np.ones((NPAD, E + 1), np.float32)
        qp[:nc_rows, 0:E] = q_full[p]
        qp[nc_rows:, 0:E] = 0.0
        qw = np.ascontiguousarray(
            qp.reshape(NT, 128, E + 1).transpose(1, 0, 2).astype(NP_BF16))

        # users per padded row; pads take the last real user (wt=0 -> no-op)
        u = np.empty(NPAD, np.int64)
        u[:nc_rows] = users_s[lo:hi]
        u[nc_rows:] = u[nc_rows - 1] if nc_rows > 0 else 0

        # bank = tile where a user first appears; slot = rank within that bank
        first_tile = {}
        slot = {}
        bank_counts = [0] * NT
        for i in range(NPAD):
            uu = u[i]
            if uu not in first_tile:
                t = i // 128
                first_tile[uu] = t
                slot[uu] = bank_counts[t]
                bank_counts[t] += 1
        assert max(bank_counts) <= 128, f"bank overflow {max(bank_counts)}"

        oh_own = np.zeros((NT, 128, 128), np.float32)
        oh_nxt = np.zeros((NT, 128, 128), np.float32)
        for i in range(NPAD):
            t, k = divmod(i, 128)
            uu = u[i]
            ft = first_tile[uu]
            if ft == t:
                oh_own[t, k, slot[uu]] = 1.0
            else:
                # sorted rows: a user spans at most 2 consecutive tiles
                assert ft == t - 1, (ft, t)
                oh_nxt[ft, k, slot[uu]] = 1.0
        ohT_own = oh_own.transpose(0, 2, 1)
        ohT_nxt = oh_nxt.transpose(0, 2, 1)
        ohs = np.zeros((NT, 128, 512), NP_FP8)
        ohs[1:, :, 0:128] = oh_nxt[:-1]
        ohs[:, :, 128:256] = oh_own
        ohs[1:, :, 256:384] = ohT_nxt[:-1]
        ohs[:, :, 384:512] = ohT_own

        in_maps.append(
            {
                "R2": R2,
                "maskc": maskc,
                "maskw": maskw,
                "cntw": cntw,
                "qw": qw,
                "w2": w2,
                "ohs": ohs,
            }
        )
        metas.append((p, nc_rows))
    return in_maps, metas, NT


# ----------------------------------------------------------------------------
# device program
# ----------------------------------------------------------------------------

def build_program(NT, alpha=1.0, beta=1.0, gamma=1.0):
    nc = bacc.Bacc(
        "TRN2", target_bir_lowering=False, debug=False, num_devices=N_CORES
    )

    R2 = nc.dram_tensor("R2", [128, NT, SP, 128], FP8, kind="ExternalInput")
    maskc = nc.dram_tensor("maskc", [128, NT, S], BF16, kind="ExternalInput")
    maskw = nc.dram_tensor("maskw", [128, NT, S], BF16, kind="ExternalInput")
    cntw = nc.dram_tensor("cntw", [128, NT], F32, kind="ExternalInput")
    qw = nc.dram_tensor("qw", [128, NT, E + 1], BF16, kind="ExternalInput")
    w2 = nc.dram_tensor("w2", [128, 2], BF16, kind="ExternalInput")
    ohs = nc.dram_tensor("ohs", [NT, 128, 512], FP8, kind="ExternalInput")
    r_out = nc.dram_tensor("r_out", [128, NT], F32, kind="ExternalOutput")

    fast = (alpha == 1.0) and (beta == 1.0) and (gamma == 1.0)
    AF = mybir.ActivationFunctionType

    with tile.TileContext(nc) as tc:
        with (
            tc.tile_pool(name="const", bufs=1) as constp,
            tc.tile_pool(name="rpool", bufs=8) as rpool,
            tc.tile_pool(name="zsc", bufs=2) as zscp,
            tc.tile_pool(name="small", bufs=8) as small,
            tc.tile_pool(name="xpool", bufs=4) as xpool,
            tc.tile_pool(name="ohpool", bufs=6) as ohpool,
            tc.tile_pool(name="pqpool", bufs=2) as pqpool,
            tc.tile_pool(name="banks", bufs=1) as bankp,
            tc.tile_pool(name="psum_z", bufs=3, space="PSUM") as pz,
            tc.tile_pool(name="psum_seg", bufs=3, space="PSUM") as pseg,
            tc.tile_pool(name="psum_gat", bufs=2, space="PSUM") as pgat,
            nc.allow_low_precision(reason="bf16 pipeline validated offline"),
        ):
            w2_sb = constp.tile([128, 2], BF16)
            nc.sync.dma_start(w2_sb[:], w2[:, :])
            mask_sb = constp.tile([128, NT, S], BF16)
            nc.scalar.dma_start(mask_sb[:], maskc[:, :, :] if fast else maskw[:, :, :])
            qw_sb = constp.tile([128, NT, E + 1], BF16)
            nc.scalar.dma_start(qw_sb[:], qw[:, :, :])
            if not fast:
                cnt_sb = constp.tile([128, NT], F32)
                nc.scalar.dma_start(cnt_sb[:], cntw[:, :])
            wt_sb = constp.tile([128, NT], F32)
            r_sb = constp.tile([128, NT], F32)
            bank_sb = bankp.tile([128, NT, E + 1], BF16)

            bank_ps = [None] * NT
            oh_tiles = [None] * NT

            def gather(t, oht):
                """Per-row num/den for tile t; needs banks t-1,t flushed."""
                gp = pgat.tile([128, E + 1], F32)
                if t >= 1:
                    nc.tensor.matmul(
                        gp[:], oht[:, 256:384], bank_sb[:, t - 1, :],
                        start=True, stop=False,
                    )
                    nc.tensor.matmul(
                        gp[:], oht[:, 384:512], bank_sb[:, t, :],
                        start=False, stop=True,
                    )
                else:
                    nc.tensor.matmul(
                        gp[:], oht[:, 384:512], bank_sb[:, t, :],
                        start=True, stop=True,
                    )
                rec = small.tile([128, 1], F32)
                nc.vector.reciprocal(rec[:], gp[:, E : E + 1])
                pq = pqpool.tile([128, E], BF16)
                # pq = (num/den)*q with row-sum accumulated straight into r
                nc.vector.scalar_tensor_tensor(
                    pq[:], gp[:, 0:E], rec[:], qw_sb[:, t, 0:E],
                    op0=mybir.AluOpType.mult, op1=mybir.AluOpType.mult,
                    accum_out=r_sb[:, t : t + 1],
                )

            for t in range(NT):
                rt = rpool.tile([128, SP, 128], FP8)
                nc.sync.dma_start(rt[:], R2[:, t, :, :])
                oht = ohpool.tile([128, 512], FP8)
                oh_tiles[t] = oht
                nc.scalar.dma_start(oht[:], ohs[t, :, :])

                # Z [128 rows, 50] via 25 PE matmuls (contract d on partitions)
                zps = pz.tile([128, S], F32)
                for j in range(SP):
                    nc.tensor.matmul(
                        zps[:, 2 * j : 2 * j + 2], rt[:, j, :], w2_sb[:],
                        start=True, stop=True,
                    )

                wt_col = wt_sb[:, t : t + 1]
                if fast:
                    wp = zscp.tile([128, S], BF16)
                    # wt = sum_s (max(z, tau) * mask*cnt), fused in one DVE op
                    nc.vector.scalar_tensor_tensor(
                        wp[:], zps[:], TAU, mask_sb[:, t, :],
                        op0=mybir.AluOpType.max, op1=mybir.AluOpType.mult,
                        accum_out=wt_col,
                    )
                else:
                    z = zscp.tile([128, S], F32)
                    nc.vector.tensor_scalar_max(z[:], zps[:], TAU)
                    # z <- exp(alpha * ln z)   (z >= TAU > 0)
                    nc.scalar.activation(z[:], z[:], AF.Log)
                    nc.scalar.activation(z[:], z[:], AF.Exp, scale=float(alpha))
                    wp = zscp.tile([128, S], F32)
                    nc.vector.tensor_mul(wp[:], z[:], mask_sb[:, t, :])
                    a_col = small.tile([128, 1], F32)
                    nc.vector.tensor_reduce(
                        a_col[:], wp[:], axis=mybir.AxisListType.X,
                        op=mybir.AluOpType.add,
                    )
                    # wt = (A^(1/alpha) * cnt^beta)^gamma
                    #    = exp(gamma*(ln(A)/alpha + beta*ln(cnt)))
                    la = small.tile([128, 1], F32)
                    nc.scalar.activation(la[:], a_col[:], AF.Log)
                    lc = small.tile([128, 1], F32)
                    nc.scalar.activation(lc[:], cnt_sb[:, t : t + 1], AF.Log)
                    nc.vector.scalar_tensor_tensor(
                        la[:], lc[:], float(alpha * beta), la[:],
                        op0=mybir.AluOpType.mult, op1=mybir.AluOpType.add,
                    )
                    nc.scalar.activation(
                        wt_col, la[:], AF.Exp, scale=float(gamma / alpha)
                    )

                # X_t = [wt*q | wt] in ONE ACT op (qw has a ones column)
                xt = xpool.tile([128, E + 1], BF16)
                nc.scalar.mul(xt[:], qw_sb[:, t, :], wt_col)

                # leftovers of this tile into previous tile's bank (closes it)
                if t >= 1:
                    nc.tensor.matmul(
                        bank_ps[t - 1][:], oh_tiles[t][:, 0:128], xt[:],
                        start=False, stop=True,
                    )
                    nc.scalar.copy(bank_sb[:, t - 1, :], bank_ps[t - 1][:])
                ps = pseg.tile([128, E + 1], F32)
                bank_ps[t] = ps
                last = t == NT - 1
                nc.tensor.matmul(
                    ps[:], oh_tiles[t][:, 128:256], xt[:], start=True, stop=last
                )
                if last:
                    nc.scalar.copy(bank_sb[:, t, :], ps[:])

                # gather for tile t-1 (its banks t-2, t-1 are now flushed)
                if t >= 1:
                    gather(t - 1, oh_tiles[t - 1])
                    oh_tiles[t - 1] = None

            gather(NT - 1, oh_tiles[NT - 1])

            nc.sync.dma_start(r_out[:, :], r_sb[:])

    nc.compile()
    return nc


# ----------------------------------------------------------------------------
# entry point
# ----------------------------------------------------------------------------

def kernel(users, items, R_ui, mask, w, item_emb, alpha, beta, gamma,
           _return_extras=False, _trace=False):
    users = np.asarray(users, np.int64)
    items = np.asarray(items, np.int64)
    R_ui = np.asarray(R_ui, np.float32)
    mask_b = np.asarray(mask)
    mask_f = mask_b.astype(np.float32)
    w = np.asarray(w, np.float32)
    item_emb = np.asarray(item_emb, np.float32)
    al = float(np.asarray(alpha).reshape(-1)[0])
    be = float(np.asarray(beta).reshape(-1)[0])
    ga = float(np.asarray(gamma).reshape(-1)[0])

    import time as _time

    t0 = _time.perf_counter()
    in_maps, metas, NT = _preprocess(users, items, R_ui, mask_f, w, item_emb)
    t1 = _time.perf_counter()
    nc = build_program(NT, al, be, ga)
    t2 = _time.perf_counter()
    res = run_bass_kernel_spmd(
        nc, in_maps, core_ids=list(range(N_CORES)), trace=_trace
    )
    t3 = _time.perf_counter()
    print(
        f"[kernel] preprocess {t1-t0:.1f}s  build+schedule {t2-t1:.1f}s  "
        f"compile+run {t3-t2:.1f}s"
    )

    n = users.shape[0]
    r = np.empty(n, np.float32)
    for c in range(N_CORES):
        p, nc_rows = metas[c]
        shard = res.results[c]["r_out"].T.reshape(-1)[:nc_rows]
        r[p] = shard
    if _return_extras:
        return r, res
    return r


# revision 54
# speedup vs baseline: 1.0014x; 1.0014x over previous
"""CDMF segment-reduce kernel for 8 Trainium2 NeuronCores.

Strategy
--------
Host (cheap, index-only + layout + one big gather):
  * stable-sort rows by user id; cut the 100k rows into 8 shards at user
    boundaries ("expert-style sharding of user segments") so each core owns a
    disjoint user range -> no cross-core reduction needed at all.
  * pad every shard to NT*128 rows (mask=0 rows contribute exactly 0).
  * pre-gather q = item_emb[items] per shard (bf16).
  * store R transposed per tile as bf16 with the feature (d) axis on SBUF
    partitions, two seq-steps stacked per 128-partition group, so the PE can
    contract against w directly (Z = R.w as 25 tiny matmuls per tile).
  * build per-tile one-hot matrices (fp8, exact 0/1) mapping the 128 rows of
    a tile to the user-slots of a PSUM "bank" (bank b = users first seen in
    tile b), packed [seg_prev|seg_own|gat_prev|gat_own] in one 512B row.

Device (one SPMD program on 8 cores, DMA-bound ~2.5us/tile):
  * stream R2 tiles [128, 25, 128] bf16; 25 PE matmuls against w2 [128, 2]
    produce Z [128 rows, 50] in PSUM (f32 accum)
  * one DVE scalar_tensor_tensor: wt = sum_s max(Z, tau) * (mask*cnt)
  * ACT builds X = [wt*q | wt] (bf16)
  * PE one-hot matmuls accumulate per-user [sum wt*q | sum wt] into PSUM
    banks; ACT flushes each bank to SBUF (bf16)
  * interleaved gather: transposed one-hot matmuls read num/den back per row,
    DVE multiplies by q and row-reduces; single reciprocal at the end.
"""

import numpy as np
import ml_dtypes

import concourse.bass as bass
import concourse.tile as tile
from concourse import bacc, mybir
from concourse.bass_utils import run_bass_kernel_spmd

N_CORES = 8
TAU = 0.01
S = 50          # seq_len
D = 64          # n_features
E = 128         # emb_dim
SP = S // 2     # s-pairs per tile
F32 = mybir.dt.float32
BF16 = mybir.dt.bfloat16
FP8 = mybir.dt.float8e4

NP_BF16 = ml_dtypes.bfloat16
NP_FP8 = ml_dtypes.float8_e4m3fn


# ----------------------------------------------------------------------------
# host-side preprocessing
# ----------------------------------------------------------------------------

def _preprocess(users, items, R_ui, mask, w, item_emb):
    n = users.shape[0]
    perm = np.argsort(users, kind="stable")
    users_s = users[perm]

    # shard cuts at user boundaries
    cuts = [0]
    for c in range(1, N_CORES):
        t = round(c * n / N_CORES)
        while 0 < t < n and users_s[t] == users_s[t - 1]:
            t += 1
        cuts.append(min(t, n))
    cuts.append(n)
    sizes = [cuts[c + 1] - cuts[c] for c in range(N_CORES)]
    NT = max(2, int(np.ceil(max(sizes) / 128)))
    NPAD = NT * 128

    q_full = item_emb[items]  # [n, E]

    # w2 [128, 2]: column 0 contracts even s (partitions 0:64), column 1 odd s
    w2 = np.zeros((128, 2), NP_BF16)
    w2[0:D, 0] = w
    w2[D:128, 1] = w

    in_maps = []
    metas = []
    for c in range(N_CORES):
        lo, hi = cuts[c], cuts[c + 1]
        nc_rows = hi - lo
        p = perm[lo:hi]

        Rp = np.zeros((NPAD, S, D), np.float32)
        Rp[:nc_rows] = R_ui[p]
        # [NT,128,SP,2,D] -> [(2,D)=128, NT, SP, 128row]
        R2 = np.ascontiguousarray(
            Rp.reshape(NT, 128, SP, 2, D).transpose(3, 4, 0, 2, 1)
            .reshape(128, NT, SP, 128).astype(NP_FP8)
        )

        mk = np.zeros((NPAD, S), np.float32)
        mk[:nc_rows] = mask[p]
        maskw = mk.reshape(NT, 128, S).transpose(1, 0, 2)
        cntw = maskw.sum(-1)  # [128, NT]
        # fast path (alpha=beta=gamma=1): wt = (sum_s mask*Wv) * cnt, so
        # pre-scaling the mask by cnt lets one fused op produce wt directly
        maskc = np.ascontiguousarray(
            (maskw * cntw[:, :, None]).astype(NP_BF16))
        maskw = np.ascontiguousarray(maskw.astype(NP_BF16))
        cntw = np.ascontiguousarray(cntw.astype(np.float32))

        # q with a ones column appended so ONE ACT op builds X = [wt*q | wt]
        qp = np.ones((NPAD, E + 1), np.float32)
        qp[:nc_rows, 0:E] = q_full[p]
        qp[nc_rows:, 0:E] = 0.0
        qw = np.ascontiguousarray(
            qp.reshape(NT, 128, E + 1).transpose(1, 0, 2).astype(NP_BF16))

        # users per padded row; pads take the last real user (wt=0 -> no-op)
        u = np.empty(NPAD, np.int64)
        u[:nc_rows] = users_s[lo:hi]
        u[nc_rows:] = u[nc_rows - 1] if nc_rows > 0 else 0

        # bank = tile where a user first appears; slot = rank within that bank
        first_tile = {}
        slot = {}
        bank_counts = [0] * NT
        for i in range(NPAD):
            uu = u[i]
            if uu not in first_tile:
                t = i // 128
                first_tile[uu] = t
                slot[uu] = bank_counts[t]
                bank_counts[t] += 1
        assert max(bank_counts) <= 128, f"bank overflow {max(bank_counts)}"

        oh_own = np.zeros((NT, 128, 128), np.float32)
        oh_nxt = np.zeros((NT, 128, 128), np.float32)
        for i in range(NPAD):
            t, k = divmod(i, 128)
            uu = u[i]
            ft = first_tile[uu]
            if ft == t:
                oh_own[t, k, slot[uu]] = 1.0
            else:
                # sorted rows: a user spans at most 2 consecutive tiles
                assert ft == t - 1, (ft, t)
                oh_nxt[ft, k, slot[uu]] = 1.0
        ohT_own = oh_own.transpose(0, 2, 1)
        ohT_nxt = oh_nxt.transpose(0, 2, 1)
        ohs = np.zeros((NT, 128, 512), NP_FP8)
        ohs[1:, :, 0:128] = oh_nxt[:-1]
        ohs[:, :, 128:256] = oh_own
        ohs[1:, :, 256:384] = ohT_nxt[:-1]
        ohs[:, :, 384:512] = ohT_own

        in_maps.append(
            {
                "R2": R2,
                "maskc": maskc,
                "maskw": maskw,
                "cntw": cntw,
                "qw": qw,
                "w2": w2,
                "ohs": ohs,
            }
        )
        metas.append((p, nc_rows))
    return in_maps, metas, NT


# ----------------------------------------------------------------------------
# device program
# ----------------------------------------------------------------------------

def build_program(NT, alpha=1.0, beta=1.0, gamma=1.0):
    nc = bacc.Bacc(
        "TRN2", target_bir_lowering=False, debug=False, num_devices=N_CORES
    )

    R2 = nc.dram_tensor("R2", [128, NT, SP, 128], FP8, kind="ExternalInput")
    maskc = nc.dram_tensor("maskc", [128, NT, S], BF16, kind="ExternalInput")
    maskw = nc.dram_tensor("maskw", [128, NT, S], BF16, kind="ExternalInput")
    cntw = nc.dram_tensor("cntw", [128, NT], F32, kind="ExternalInput")
    qw = nc.dram_tensor("qw", [128, NT, E + 1], BF16, kind="ExternalInput")
    w2 = nc.dram_tensor("w2", [128, 2], BF16, kind="ExternalInput")
    ohs = nc.dram_tensor("ohs", [NT, 128, 512], FP8, kind="ExternalInput")
    r_out = nc.dram_tensor("r_out", [128, NT], F32, kind="ExternalOutput")

    fast = (alpha == 1.0) and (beta == 1.0) and (gamma == 1.0)
    AF = mybir.ActivationFunctionType

    with tile.TileContext(nc) as tc:
        with (
            tc.tile_pool(name="const", bufs=1) as constp,
            tc.tile_pool(name="rpool", bufs=12) as rpool,
            tc.tile_pool(name="zsc", bufs=2) as zscp,
            tc.tile_pool(name="small", bufs=8) as small,
            tc.tile_pool(name="xpool", bufs=4) as xpool,
            tc.tile_pool(name="ohpool", bufs=10) as ohpool,
            tc.tile_pool(name="pqpool", bufs=4) as pqpool,
            tc.tile_pool(name="banks", bufs=1) as bankp,
            tc.tile_pool(name="psum_z", bufs=3, space="PSUM") as pz,
            tc.tile_pool(name="psum_seg", bufs=3, space="PSUM") as pseg,
            tc.tile_pool(name="psum_gat", bufs=2, space="PSUM") as pgat,
            nc.allow_low_precision(reason="bf16 pipeline validated offline"),
        ):
            w2_sb = constp.tile([128, 2], BF16)
            nc.sync.dma_start(w2_sb[:], w2[:, :])
            mask_sb = constp.tile([128, NT, S], BF16)
            nc.scalar.dma_start(mask_sb[:], maskc[:, :, :] if fast else maskw[:, :, :])
            qw_sb = constp.tile([128, NT, E + 1], BF16)
            nc.scalar.dma_start(qw_sb[:], qw[:, :, :])
            if not fast:
                cnt_sb = constp.tile([128, NT], F32)
                nc.scalar.dma_start(cnt_sb[:], cntw[:, :])
            wt_sb = constp.tile([128, NT], F32)
            r_sb = constp.tile([128, NT], F32)
            bank_sb = bankp.tile([128, NT, E + 1], BF16)

            bank_ps = [None] * NT
            oh_tiles = [None] * NT

            def gather(t, oht):
                """Per-row num/den for tile t; needs banks t-1,t flushed."""
                gp = pgat.tile([128, E + 1], F32)
                if t >= 1:
                    nc.tensor.matmul(
                        gp[:], oht[:, 256:384], bank_sb[:, t - 1, :],
                        start=True, stop=False,
                    )
                    nc.tensor.matmul(
                        gp[:], oht[:, 384:512], bank_sb[:, t, :],
                        start=False, stop=True,
                    )
                else:
                    nc.tensor.matmul(
                        gp[:], oht[:, 384:512], bank_sb[:, t, :],
                        start=True, stop=True,
                    )
                rec = small.tile([128, 1], F32)
                nc.vector.reciprocal(rec[:], gp[:, E : E + 1])
                pq = pqpool.tile([128, E], BF16)
                # pq = (num/den)*q with row-sum accumulated straight into r
                nc.vector.scalar_tensor_tensor(
                    pq[:], gp[:, 0:E], rec[:], qw_sb[:, t, 0:E],
                    op0=mybir.AluOpType.mult, op1=mybir.AluOpType.mult,
                    accum_out=r_sb[:, t : t + 1],
                )

            for t in range(NT):
                rt = rpool.tile([128, SP, 128], FP8)
                nc.sync.dma_start(rt[:], R2[:, t, :, :])
                oht = ohpool.tile([128, 512], FP8)
                oh_tiles[t] = oht
                nc.scalar.dma_start(oht[:], ohs[t, :, :])

                # Z [128 rows, 50] via 25 PE matmuls (contract d on partitions)
                zps = pz.tile([128, S], F32)
                for j in range(SP):
                    nc.tensor.matmul(
                        zps[:, 2 * j : 2 * j + 2], rt[:, j, :], w2_sb[:],
                        start=True, stop=True,
                    )

                wt_col = wt_sb[:, t : t + 1]
                if fast:
                    wp = zscp.tile([128, S], BF16)
                    # wt = sum_s (max(z, tau) * mask*cnt), fused in one DVE op
                    nc.vector.scalar_tensor_tensor(
                        wp[:], zps[:], TAU, mask_sb[:, t, :],
                        op0=mybir.AluOpType.max, op1=mybir.AluOpType.mult,
                        accum_out=wt_col,
                    )
                else:
                    z = zscp.tile([128, S], F32)
                    nc.vector.tensor_scalar_max(z[:], zps[:], TAU)
                    # z <- exp(alpha * ln z)   (z >= TAU > 0)
                    nc.scalar.activation(z[:], z[:], AF.Log)
                    nc.scalar.activation(z[:], z[:], AF.Exp, scale=float(alpha))
                    wp = zscp.tile([128, S], F32)
                    nc.vector.tensor_mul(wp[:], z[:], mask_sb[:, t, :])
                    a_col = small.tile([128, 1], F32)
                    nc.vector.tensor_reduce(
                        a_col[:], wp[:], axis=mybir.AxisListType.X,
                        op=mybir.AluOpType.add,
                    )
                    # wt = (A^(1/alpha) * cnt^beta)^gamma
                    #    = exp(gamma*(ln(A)/alpha + beta*ln(cnt)))
                    la = small.tile([128, 1], F32)
                    nc.scalar.activation(la[:], a_col[:], AF.Log)
                    lc = small.tile([128, 1], F32)
                    nc.scalar.activation(lc[:], cnt_sb[:, t : t + 1], AF.Log)
                    nc.vector.scalar_tensor_tensor(
                        la[:], lc[:], float(alpha * beta), la[:],
                        op0=mybir.AluOpType.mult, op1=mybir.AluOpType.add,
                    )
                    nc.scalar.activation(
                        wt_col, la[:], AF.Exp, scale=float(gamma / alpha)
                    )

                # X_t = [wt*q | wt] in ONE ACT op (qw has a ones column)
                xt = xpool.tile([128, E + 1], BF16)
                nc.scalar.mul(xt[:], qw_sb[:, t, :], wt_col)

                # leftovers of this tile into previous tile's bank (closes it)
                if t >= 1:
                    nc.tensor.matmul(
                        bank_ps[t - 1][:], oh_tiles[t][:, 0:128], xt[:],
                        start=False, stop=True,
                    )
                    nc.scalar.copy(bank_sb[:, t - 1, :], bank_ps[t - 1][:])
                ps = pseg.tile([128, E + 1], F32)
                bank_ps[t] = ps
                last = t == NT - 1
                nc.tensor.matmul(
                    ps[:], oh_tiles[t][:, 128:256], xt[:], start=True, stop=last
                )
                if last:
                    nc.scalar.copy(bank_sb[:, t, :], ps[:])

                # gather for tile t-1 (its banks t-2, t-1 are now flushed)
                if t >= 1:
                    gather(t - 1, oh_tiles[t - 1])
                    oh_tiles[t - 1] = None

            gather(NT - 1, oh_tiles[NT - 1])

            nc.sync.dma_start(r_out[:, :], r_sb[:])

    nc.compile()
    return nc


# ----------------------------------------------------------------------------
# entry point
# ----------------------------------------------------------------------------

def kernel(users, items, R_ui, mask, w, item_emb, alpha, beta, gamma,
           _return_extras=False, _trace=False):
    users = np.asarray(users, np.int64)
    items = np.asarray(items, np.int64)
    R_ui = np.asarray(R_ui, np.float32)
    mask_b = np.asarray(mask)
    mask_f = mask_b.astype(np.float32)
    w = np.asarray(w, np.float32)
    item_emb = np.asarray(item_emb, np.float32)
    al = float(np.asarray(alpha).reshape(-1)[0])
    be = float(np.asarray(beta).reshape(-1)[0])
    ga = float(np.asarray(gamma).reshape(-1)[0])

    import time as _time

    t0 = _time.perf_counter()
    in_maps, metas, NT = _preprocess(users, items, R_ui, mask_f, w, item_emb)
    t1 = _time.perf_counter()
    nc = build_program(NT, al, be, ga)
    t2 = _time.perf_counter()
    res = run_bass_kernel_spmd(
        nc, in_maps, core_ids=list(range(N_CORES)), trace=_trace
    )
    t3 = _time.perf_counter()
    print(
        f"[kernel] preprocess {t1-t0:.1f}s  build+schedule {t2-t1:.1f}s  "
        f"compile+run {t3-t2:.1f}s"
    )

    n = users.shape[0]
    r = np.empty(n, np.float32)
    for c in range(N_CORES):
        p, nc_rows = metas[c]
        shard = res.results[c]["r_out"].T.reshape(-1)[:nc_rows]
        r[p] = shard
    if _return_extras:
        return r, res
    return r
